# revision 25
# baseline (speedup 1.0000x reference)
"""Trainium2 Bass kernel for nn_CompositionBlock (gnn_message_passing).

Data-parallel over batch B=8 across 8 NeuronCores (one sample per core).
Transposed layout: components (p/o) on partitions of the big bilinear
intermediates; TensorE does the contractions AND both reduction passes (via
tiny selection-matrix matmuls). PSUM evacuation is split across ScalarE
(copy to f16) / GpSimd (direct multiply) / VectorE (direct multiply), with
VectorE doing the remaining multiplies in 2x f16 mode. Stage-1-critical
DMAs (w1, tokT, depT8) are issued first; stage-2 weights stream in during
stage-1 compute.
"""

import copy
import json

import numpy as np

B, S, T, D, P = 8, 256, 128, 64, 128
NCORES = 8
JT = S // 128  # token tiles per core


# ----------------------------------------------------------------------------
# Compat: the walrus build in this container accepts at most one sync-wait on
# CTRL-class instructions, but TileContext's tail drain packs several. Split
# any multi-wait instruction into a chain of single-wait clones.
# ----------------------------------------------------------------------------
def _split_multiwait_bir(bir_json_bytes: bytes) -> bytes:
    bir = json.loads(bir_json_bytes)
    for func in bir.get("functions", []):
        for bb in func.get("blocks", []):
            new_instructions = []
            for ins in bb.get("instructions", []):
                si = ins.get("sync_info") or {}
                waits = si.get("on_wait") or []
                if len(waits) > 1:
                    # hoist all but the last wait onto same-engine NoOps,
                    # executed in order by the engine's sequencer just before
                    # the original instruction
                    for i, w in enumerate(waits[:-1]):
                        new_instructions.append({
                            "debug": ins.get("debug", 0),
                            "engine": ins["engine"],
                            "ins": [],
                            "name": f"{ins['name']}_w{i}",
                            "opcode": "NoOp",
                            "outs": [],
                            "sync_info": {"on_wait": [w], "on_update": []},
                        })
                    ins["sync_info"] = {
                        "on_wait": [waits[-1]],
                        "on_update": si.get("on_update") or [],
                    }
                new_instructions.append(ins)
            bb["instructions"] = new_instructions
    return json.dumps(bir).encode()


def _install_compat():
    import concourse.bass_utils as bu

    if getattr(bu.compile_bir_kernel, "_multiwait_patched", False):
        return
    orig = bu.compile_bir_kernel

    def patched(bir_json, tmpdir, neff_name="file.neff"):
        return orig(_split_multiwait_bir(bir_json), tmpdir, neff_name)

    patched._multiwait_patched = True
    bu.compile_bir_kernel = patched
    try:
        import concourse.bass2jax as b2j

        if getattr(b2j, "compile_bir_kernel", None) is not None:
            b2j.compile_bir_kernel = patched
    except ImportError:
        pass


_NC_CACHE = {}


def build_nc():
    if "nc" in _NC_CACHE:
        return _NC_CACHE["nc"]
    import concourse.bass as bass
    import concourse.tile as tile
    from concourse import mybir
    from concourse.masks import make_identity

    f32 = mybir.dt.float32
    f16 = mybir.dt.float16
    Alu = mybir.AluOpType
    Act = mybir.ActivationFunctionType

    nc = bass.Bass(trn_type="TRN2")

    tokT_d = nc.dram_tensor("tokT", [T, S], f16, kind="ExternalInput")
    tokT8_d = nc.dram_tensor("tokT8", [128, 8 * 512], f16, kind="ExternalInput")
    depT8_d = nc.dram_tensor("depT8", [128, 4 * 512], f16, kind="ExternalInput")
    w1t_d = nc.dram_tensor("w1t", [T, 64 * 128], f16, kind="ExternalInput")
    w2t_d = nc.dram_tensor("w2t", [P, 128 * 128], f16, kind="ExternalInput")
    # cf32 columns: [bdep | bcomp | base | headsf0 | headsf1 | wr0 | wr1 | c0(T)]
    cf32_d = nc.dram_tensor("cf32", [128, 7 + T], f32, kind="ExternalInput")
    # cf16 columns: [iota(S) | red(64)]
    cf16_d = nc.dram_tensor("cf16", [128, S + 64], f16, kind="ExternalInput")
    out_d = nc.dram_tensor("out", [S, T], f32, kind="ExternalOutput")

    with tile.TileContext(nc) as tc:
        with (
            tc.tile_pool(name="consts", bufs=1) as consts,
            tc.tile_pool(name="weights", bufs=1) as weights,
            tc.tile_pool(name="work", bufs=6) as work,
            tc.tile_pool(name="keep", bufs=1) as keep,
            tc.tile_pool(name="psmm", bufs=5, space="PSUM") as psmm,
            tc.tile_pool(name="pstde", bufs=1, space="PSUM") as pstde,
            tc.tile_pool(name="pscomp", bufs=1, space="PSUM") as pscomp,
            tc.tile_pool(name="psfin", bufs=1, space="PSUM") as psfin,
        ):
            # ---- stage-1-critical DMAs first, chunked so compute can start
            # as soon as the first chunk lands ----
            w1_sb = []
            for i in range(4):
                t = weights.tile([128, 2048], f16, name=f"w1_{i}", tag=f"w1_{i}")
                nc.sync.dma_start(out=t, in_=w1t_d[:, i * 2048 : (i + 1) * 2048])
                w1_sb.append(t)

            tokT_sb = consts.tile([128, S], f16)
            nc.scalar.dma_start(out=tokT_sb, in_=tokT_d[:, :])
            depT8_sb = weights.tile([128, 4 * 512], f16, name="dep8", tag="dep8")
            nc.scalar.dma_start(out=depT8_sb, in_=depT8_d[:, :])

            # merged small consts — one DMA per ring, ahead of stage-2 bulk
            # (red_sb gates every reduce matmul)
            cf32 = consts.tile([128, 7 + T], f32, name="cf32", tag="cf32")
            nc.scalar.dma_start(out=cf32, in_=cf32_d[:, :])
            cf16 = consts.tile([128, S + 64], f16, name="cf16", tag="cf16")
            nc.sync.dma_start(out=cf16, in_=cf16_d[:, :])
            bdep_c = cf32[:, 0:1]
            bcomp_c = cf32[:, 1:2]
            base_c = cf32[:, 2:3]
            headsf_t = [cf32[:, 3 + jt : 4 + jt] for jt in range(JT)]
            wr_t = [cf32[:, 5 + jt : 6 + jt] for jt in range(JT)]
            c0_b = cf32[:, 7 : 7 + T]
            iota_b = cf16[:, 0:S]

            def red_sel(par):
                return cf16[:, S + 32 * par : S + 32 * par + 32]

            ident16 = consts.tile([128, 128], f16)
            make_identity(nc, ident16)

            # ---- stage-2 weights, chunked, queued on the same hardware DGE
            # rings BEHIND everything stage-1 needs (per-ring FIFO =
            # priority). Never use gpsimd triggers for big transfers: they
            # take the software-DGE path whose huge packets starve the hw
            # rings. ----
            tokT8_sb = []
            for i in range(2):
                t = weights.tile([128, 2048], f16, name=f"tok8_{i}", tag=f"tok8_{i}")
                eng = [nc.sync, nc.scalar][i % 2]
                eng.dma_start(out=t, in_=tokT8_d[:, i * 2048 : (i + 1) * 2048])
                tokT8_sb.append(t)
            w2_sb = []
            for i in range(4):
                t = weights.tile([128, 4096], f16, name=f"w2_{i}", tag=f"w2_{i}")
                eng = [nc.sync, nc.scalar][i % 2]
                eng.dma_start(out=t, in_=w2t_d[:, i * 4096 : (i + 1) * 4096])
                w2_sb.append(t)

            # per-tile handling: "SD" ScalarE evac + DVE mult, "SG" ScalarE
            # evac + GpSimd mult (GpSimd can't read PSUM), "DV" DVE direct
            EVAC = ["SD", "SG", "DV", "SD", "SG", "DV",
                    "SD", "SG", "DV", "SG", "DV", "DV"]

            def mm_stage(n_cp, lhs_tiles, lhs_cpt, rhs_sb, mult_fn, mult_period,
                         acc_ps, acc_group, prod_name):
                """Software-pipelined mains -> evac/mult -> reduce."""
                LAG = 3
                prods = {}
                for step in range(n_cp + LAG):
                    if step < n_cp:
                        cp = step
                        c0i, c1i = 2 * cp, 2 * cp + 1
                        ps = psmm.tile([128, 512], f32, name="mm", tag="mm")
                        for k, c in enumerate((c0i, c1i)):
                            nc.tensor.matmul(
                                ps[:, k * 256 : (k + 1) * 256],
                                lhs_tiles[c // lhs_cpt][
                                    :, (c % lhs_cpt) * 128 : (c % lhs_cpt + 1) * 128
                                ],
                                rhs_sb,
                            )
                        prod = work.tile(
                            [128, 512], f16, name=prod_name, tag=prod_name
                        )
                        msl = mult_fn((c0i % mult_period) // 2)
                        kind = EVAC[cp % 12]
                        if kind == "SD" or kind == "SG":
                            praw = work.tile(
                                [128, 512], f16, name="praw", tag="praw"
                            )
                            nc.scalar.copy(out=praw, in_=ps)
                            eng = nc.vector if kind == "SD" else nc.gpsimd
                            eng.tensor_tensor(
                                out=prod, in0=praw, in1=msl, op=Alu.mult
                            )
                        else:
                            nc.vector.tensor_tensor(
                                out=prod, in0=ps, in1=msl, op=Alu.mult
                            )
                        prods[cp] = prod
                    if step >= LAG:
                        cp = step - LAG
                        prod = prods.pop(cp)
                        for k, c in enumerate((2 * cp, 2 * cp + 1)):
                            a = c // acc_group
                            g = a // 2
                            nc.tensor.matmul(
                                acc_ps[32 * g : 32 * g + 32, :],
                                red_sel(a % 2),
                                prod[:, k * 256 : (k + 1) * 256],
                                start=(c % (2 * acc_group) == 0),
                                stop=(c % (2 * acc_group) == 2 * acc_group - 1),
                                tile_position=(0, 32 * g),
                            )

            # ---- stage 1 ----
            tde_ps = pstde.tile([128, S], f32)
            mm_stage(
                32, w1_sb, 16, tokT_sb,
                lambda m: depT8_sb[:, m * 512 : (m + 1) * 512],
                8, tde_ps, 8, "prod1",
            )
            hT = keep.tile([128, S], f16)
            nc.scalar.activation(hT, tde_ps, Act.Tanh, bias=bdep_c)

            # soh tiles depend only on consts — built here (DVE has slack
            # at the stage boundary) so the tail after stage 2 is short
            soh = []
            for jt in range(JT):
                s = keep.tile([128, S], f16, name=f"soh{jt}", tag=f"soh{jt}")
                nc.vector.tensor_scalar(
                    out=s, in0=iota_b, scalar1=headsf_t[jt], scalar2=wr_t[jt],
                    op0=Alu.is_equal, op1=Alu.mult,
                )
                soh.append(s)

            # ---- stage 2 ----
            comp_ps = pscomp.tile([128, S], f32)
            mm_stage(
                64, w2_sb, 32, hT,
                lambda m: tokT8_sb[m // 4][:, (m % 4) * 512 : (m % 4 + 1) * 512],
                16, comp_ps, 16, "prod2",
            )

            specT = work.tile([128, S], f16, name="specT", tag="specT")
            nc.scalar.activation(specT, comp_ps, Act.Tanh, bias=bcomp_c)
            deltaT = keep.tile([128, S], f16)
            nc.vector.tensor_scalar(
                out=deltaT, in0=specT, scalar1=base_c, scalar2=None,
                op0=Alu.subtract,
            )

            # transpose deltaT -> delta[j, o] per token tile, build soh, final
            delta_sb = []
            for jt in range(JT):
                dps = psmm.tile([128, 512], f16, name="mm", tag="mm")
                nc.tensor.transpose(
                    dps[:, 0:128], deltaT[:, jt * 128 : (jt + 1) * 128], ident16
                )
                dsb = keep.tile([128, 128], f16, name=f"delta{jt}", tag=f"delta{jt}")
                nc.scalar.copy(dsb, dps[:, 0:128])
                delta_sb.append(dsb)

            fin_ps = psfin.tile([128, S], f32)
            for ic in range(2):
                for jt in range(JT):
                    nc.tensor.matmul(
                        fin_ps[:, ic * 128 : (ic + 1) * 128],
                        soh[jt][:, ic * 128 : (ic + 1) * 128],
                        delta_sb[jt],
                        start=(jt == 0),
                        stop=(jt == JT - 1),
                    )
            for ic in range(2):
                outsb = work.tile([128, T], f32, name="outsb", tag="outsb")
                nc.vector.tensor_add(
                    outsb, fin_ps[:, ic * 128 : (ic + 1) * 128], c0_b
                )
                nc.sync.dma_start(
                    out=out_d[ic * 128 : (ic + 1) * 128, :], in_=outsb
                )

    _NC_CACHE["nc"] = nc
    return nc


def prep_core_inputs(token_embeddings, dep_embeddings, dep_heads,
                     W_dep, b_dep, W_comp, b_comp, W_red, b_red):
    f32 = np.float32
    f16 = np.float16
    tok = np.asarray(token_embeddings, dtype=f32)
    dep = np.asarray(dep_embeddings, dtype=f32)
    heads = np.asarray(dep_heads)
    W_dep = np.asarray(W_dep, dtype=f32)
    b_dep = np.asarray(b_dep, dtype=f32)
    W_comp = np.asarray(W_comp, dtype=f32)
    b_comp = np.asarray(b_comp, dtype=f32)
    wr = np.asarray(W_red, dtype=f32)[0]
    b_red = np.asarray(b_red, dtype=f32)

    # w1t[(a,b), t, (p'*8+d')] = W_dep[16a+p', t, 8b+d']
    X = W_dep.reshape(8, 16, T, 8, 8)            # [a, p', t, b, d']
    w1t = np.ascontiguousarray(
        X.transpose(2, 0, 3, 1, 4).reshape(T, 64 * 128)
    ).astype(f16)                                # [t, ((a,b), (p',d'))]
    # w2t[(a,b), p, (o'*8+t')] = W_comp[16a+o', 8b+t', p]
    Y = W_comp.reshape(8, 16, 16, 8, P)          # [a, o', b, t', p]
    w2t = np.ascontiguousarray(
        Y.transpose(4, 0, 2, 1, 3).reshape(P, 128 * 128)
    ).astype(f16)                                # [p, ((a,b), (o',t'))]
    # red[par][r, m] = (m == 16*par + r//8)
    r = np.arange(128)
    red = np.zeros((128, 2, 32), dtype=f16)
    for par in range(2):
        red[r, par, 16 * par + r // 8] = 1.0
    red = red.reshape(128, 64)

    base = np.tanh(b_comp)
    c0 = (base * wr.sum() + b_red[0]).astype(f32)
    headsf = heads.astype(f32).reshape(B, JT, 128)
    wr_cols = wr.reshape(JT, 128)

    # cf16: [iota broadcast (S) | red (64)]
    cf16 = np.empty((128, S + 64), dtype=f16)
    cf16[:, 0:S] = np.arange(S, dtype=f16)[None, :]
    cf16[:, S:] = red

    shared = {"w1t": w1t, "w2t": w2t, "cf16": cf16}
    in_maps = []
    for c in range(NCORES):
        tokc = tok[c]                             # [S, T]
        depc = dep[c]                             # [S, D]
        tokTc = np.ascontiguousarray(tokc.T)      # [T, S]
        tokT8 = np.empty((16, 128, S), dtype=f32)
        for b in range(16):
            tokT8[b] = np.tile(tokTc[8 * b : 8 * b + 8, :], (16, 1))
        depT = depc.T                             # [D, S]
        depT8 = np.empty((8, 128, S), dtype=f32)
        for b in range(8):
            depT8[b] = np.tile(depT[8 * b : 8 * b + 8, :], (16, 1))
        m = dict(shared)
        m["tokT"] = tokTc.astype(f16)
        m["tokT8"] = np.ascontiguousarray(
            tokT8.reshape(8, 2, 128, S).transpose(0, 2, 1, 3).reshape(8, 128, 2 * S)
            .transpose(1, 0, 2).reshape(128, 8 * 512)
        ).astype(f16)
        m["depT8"] = np.ascontiguousarray(
            depT8.reshape(4, 2, 128, S).transpose(0, 2, 1, 3).reshape(4, 128, 2 * S)
            .transpose(1, 0, 2).reshape(128, 4 * 512)
        ).astype(f16)
        # cf32: [bdep | bcomp | base | headsf0 | headsf1 | wr0 | wr1 | c0(T)]
        cf32 = np.empty((128, 7 + T), dtype=f32)
        cf32[:, 0] = b_dep
        cf32[:, 1] = b_comp
        cf32[:, 2] = base
        cf32[:, 3] = headsf[c, 0]
        cf32[:, 4] = headsf[c, 1]
        cf32[:, 5] = wr_cols[0]
        cf32[:, 6] = wr_cols[1]
        cf32[:, 7:] = c0[None, :]
        m["cf32"] = cf32
        in_maps.append(m)
    return in_maps


def kernel(**inputs) -> np.ndarray:
    _install_compat()
    from concourse.bass_utils import run_bass_kernel_spmd

    nc = build_nc()
    in_maps = prep_core_inputs(**inputs)
    res = run_bass_kernel_spmd(nc, in_maps, core_ids=list(range(NCORES)))
    out = np.stack([res.results[c]["out"] for c in range(NCORES)], axis=0)
    return out.astype(np.float32)


# aliases used by test harness
_build_nc = build_nc
_prep_core_inputs = prep_core_inputs


# revision 26
# speedup vs baseline: 1.0821x; 1.0821x over previous
"""Trainium2 Bass kernel for nn_CompositionBlock (gnn_message_passing).

Data-parallel over batch B=8 across 8 NeuronCores (one sample per core).
Transposed layout: components (p/o) on partitions of the big bilinear
intermediates; TensorE does the contractions AND both reduction passes (via
tiny selection-matrix matmuls). PSUM evacuation is split across ScalarE
(copy to f16) / GpSimd (direct multiply) / VectorE (direct multiply), with
VectorE doing the remaining multiplies in 2x f16 mode. Stage-1-critical
DMAs (w1, tokT, depT8) are issued first; stage-2 weights stream in during
stage-1 compute.
"""

import copy
import json

import numpy as np

B, S, T, D, P = 8, 256, 128, 64, 128
NCORES = 8
JT = S // 128  # token tiles per core


# ----------------------------------------------------------------------------
# Compat: the walrus build in this container accepts at most one sync-wait on
# CTRL-class instructions, but TileContext's tail drain packs several. Split
# any multi-wait instruction into a chain of single-wait clones.
# ----------------------------------------------------------------------------
def _split_multiwait_bir(bir_json_bytes: bytes) -> bytes:
    bir = json.loads(bir_json_bytes)
    for func in bir.get("functions", []):
        for bb in func.get("blocks", []):
            new_instructions = []
            for ins in bb.get("instructions", []):
                si = ins.get("sync_info") or {}
                waits = si.get("on_wait") or []
                if len(waits) > 1:
                    # hoist all but the last wait onto same-engine NoOps,
                    # executed in order by the engine's sequencer just before
                    # the original instruction
                    for i, w in enumerate(waits[:-1]):
                        new_instructions.append({
                            "debug": ins.get("debug", 0),
                            "engine": ins["engine"],
                            "ins": [],
                            "name": f"{ins['name']}_w{i}",
                            "opcode": "NoOp",
                            "outs": [],
                            "sync_info": {"on_wait": [w], "on_update": []},
                        })
                    ins["sync_info"] = {
                        "on_wait": [waits[-1]],
                        "on_update": si.get("on_update") or [],
                    }
                new_instructions.append(ins)
            bb["instructions"] = new_instructions
    return json.dumps(bir).encode()


def _install_compat():
    import concourse.bass_utils as bu

    if getattr(bu.compile_bir_kernel, "_multiwait_patched", False):
        return
    orig = bu.compile_bir_kernel

    def patched(bir_json, tmpdir, neff_name="file.neff"):
        return orig(_split_multiwait_bir(bir_json), tmpdir, neff_name)

    patched._multiwait_patched = True
    bu.compile_bir_kernel = patched
    try:
        import concourse.bass2jax as b2j

        if getattr(b2j, "compile_bir_kernel", None) is not None:
            b2j.compile_bir_kernel = patched
    except ImportError:
        pass


_NC_CACHE = {}


def build_nc():
    if "nc" in _NC_CACHE:
        return _NC_CACHE["nc"]
    import concourse.bass as bass
    import concourse.tile as tile
    from concourse import mybir
    from concourse.masks import make_identity

    f32 = mybir.dt.float32
    f16 = mybir.dt.float16
    Alu = mybir.AluOpType
    Act = mybir.ActivationFunctionType

    nc = bass.Bass(trn_type="TRN2")

    tokT_d = nc.dram_tensor("tokT", [T, S], f16, kind="ExternalInput")
    tokT8_d = nc.dram_tensor("tokT8", [128, 8 * 512], f16, kind="ExternalInput")
    depT8_d = nc.dram_tensor("depT8", [128, 4 * 512], f16, kind="ExternalInput")
    w1t_d = nc.dram_tensor("w1t", [T, 64 * 128], f16, kind="ExternalInput")
    w2t_d = nc.dram_tensor("w2t", [P, 128 * 128], f16, kind="ExternalInput")
    # cf32 columns: [bdep | bcomp | base | headsf0 | headsf1 | wr0 | wr1 | c0(T)]
    cf32_d = nc.dram_tensor("cf32", [128, 7 + T], f32, kind="ExternalInput")
    # cf16 columns: [iota(S) | red(64)]
    cf16_d = nc.dram_tensor("cf16", [128, S + 64], f16, kind="ExternalInput")
    out_d = nc.dram_tensor("out", [S, T], f32, kind="ExternalOutput")

    with tile.TileContext(nc) as tc:
        with (
            tc.tile_pool(name="consts", bufs=1) as consts,
            tc.tile_pool(name="weights", bufs=1) as weights,
            tc.tile_pool(name="work", bufs=6) as work,
            tc.tile_pool(name="keep", bufs=1) as keep,
            tc.tile_pool(name="psmm", bufs=5, space="PSUM") as psmm,
            tc.tile_pool(name="pstde", bufs=1, space="PSUM") as pstde,
            tc.tile_pool(name="pscomp", bufs=1, space="PSUM") as pscomp,
            tc.tile_pool(name="psfin", bufs=1, space="PSUM") as psfin,
        ):
            # ---- merged small consts ride first on each ring: they gate
            # many small ops (soh, reduces via red_sb) and are tiny ----
            cf16 = consts.tile([128, S + 64], f16, name="cf16", tag="cf16")
            nc.sync.dma_start(out=cf16, in_=cf16_d[:, :])
            cf32 = consts.tile([128, 7 + T], f32, name="cf32", tag="cf32")
            nc.scalar.dma_start(out=cf32, in_=cf32_d[:, :])

            # ---- stage-1-critical DMAs next, chunked so compute can start
            # as soon as the first chunk lands ----
            tokT_sb = consts.tile([128, S], f16)
            nc.scalar.dma_start(out=tokT_sb, in_=tokT_d[:, :])
            depT8_sb = weights.tile([128, 4 * 512], f16, name="dep8", tag="dep8")
            nc.scalar.dma_start(out=depT8_sb, in_=depT8_d[:, :])
            w1_sb = []
            for i in range(4):
                t = weights.tile([128, 2048], f16, name=f"w1_{i}", tag=f"w1_{i}")
                nc.sync.dma_start(out=t, in_=w1t_d[:, i * 2048 : (i + 1) * 2048])
                w1_sb.append(t)
            bdep_c = cf32[:, 0:1]
            bcomp_c = cf32[:, 1:2]
            base_c = cf32[:, 2:3]
            headsf_t = [cf32[:, 3 + jt : 4 + jt] for jt in range(JT)]
            wr_t = [cf32[:, 5 + jt : 6 + jt] for jt in range(JT)]
            c0_b = cf32[:, 7 : 7 + T]
            iota_b = cf16[:, 0:S]

            def red_sel(par):
                return cf16[:, S + 32 * par : S + 32 * par + 32]

            ident16 = consts.tile([128, 128], f16)
            make_identity(nc, ident16)

            # ---- stage-2 weights, chunked, queued on the same hardware DGE
            # rings BEHIND everything stage-1 needs (per-ring FIFO =
            # priority). Never use gpsimd triggers for big transfers: they
            # take the software-DGE path whose huge packets starve the hw
            # rings. ----
            tokT8_sb = []
            for i in range(2):
                t = weights.tile([128, 2048], f16, name=f"tok8_{i}", tag=f"tok8_{i}")
                eng = [nc.sync, nc.scalar][i % 2]
                eng.dma_start(out=t, in_=tokT8_d[:, i * 2048 : (i + 1) * 2048])
                tokT8_sb.append(t)
            w2_sb = []
            for i in range(4):
                t = weights.tile([128, 4096], f16, name=f"w2_{i}", tag=f"w2_{i}")
                eng = [nc.sync, nc.scalar][i % 2]
                eng.dma_start(out=t, in_=w2t_d[:, i * 4096 : (i + 1) * 4096])
                w2_sb.append(t)

            # per-tile handling: "SD" ScalarE evac + DVE mult, "SG" ScalarE
            # evac + GpSimd mult (GpSimd can't read PSUM), "DV" DVE direct
            EVAC = ["SD", "SG", "DV", "SD", "SG", "DV",
                    "SD", "SG", "DV", "SG", "DV", "DV"]

            def mm_stage(n_cp, lhs_tiles, lhs_cpt, rhs_sb, mult_fn, mult_period,
                         acc_ps, acc_group, prod_name):
                """Software-pipelined mains -> evac/mult -> reduce."""
                LAG = 3
                prods = {}
                for step in range(n_cp + LAG):
                    if step < n_cp:
                        cp = step
                        c0i, c1i = 2 * cp, 2 * cp + 1
                        ps = psmm.tile([128, 512], f32, name="mm", tag="mm")
                        for k, c in enumerate((c0i, c1i)):
                            nc.tensor.matmul(
                                ps[:, k * 256 : (k + 1) * 256],
                                lhs_tiles[c // lhs_cpt][
                                    :, (c % lhs_cpt) * 128 : (c % lhs_cpt + 1) * 128
                                ],
                                rhs_sb,
                            )
                        prod = work.tile(
                            [128, 512], f16, name=prod_name, tag=prod_name
                        )
                        msl = mult_fn((c0i % mult_period) // 2)
                        kind = EVAC[cp % 12]
                        if kind == "SD" or kind == "SG":
                            praw = work.tile(
                                [128, 512], f16, name="praw", tag="praw"
                            )
                            nc.scalar.copy(out=praw, in_=ps)
                            eng = nc.vector if kind == "SD" else nc.gpsimd
                            eng.tensor_tensor(
                                out=prod, in0=praw, in1=msl, op=Alu.mult
                            )
                        else:
                            nc.vector.tensor_tensor(
                                out=prod, in0=ps, in1=msl, op=Alu.mult
                            )
                        prods[cp] = prod
                    if step >= LAG:
                        cp = step - LAG
                        prod = prods.pop(cp)
                        for k, c in enumerate((2 * cp, 2 * cp + 1)):
                            a = c // acc_group
                            g = a // 2
                            nc.tensor.matmul(
                                acc_ps[32 * g : 32 * g + 32, :],
                                red_sel(a % 2),
                                prod[:, k * 256 : (k + 1) * 256],
                                start=(c % (2 * acc_group) == 0),
                                stop=(c % (2 * acc_group) == 2 * acc_group - 1),
                                tile_position=(0, 32 * g),
                            )

            # ---- stage 1 ----
            tde_ps = pstde.tile([128, S], f32)
            mm_stage(
                32, w1_sb, 16, tokT_sb,
                lambda m: depT8_sb[:, m * 512 : (m + 1) * 512],
                8, tde_ps, 8, "prod1",
            )
            hT = keep.tile([128, S], f16)
            nc.scalar.activation(hT, tde_ps, Act.Tanh, bias=bdep_c)

            # soh tiles depend only on consts — built here (DVE has slack
            # at the stage boundary) so the tail after stage 2 is short
            soh = []
            for jt in range(JT):
                s = keep.tile([128, S], f16, name=f"soh{jt}", tag=f"soh{jt}")
                nc.vector.tensor_scalar(
                    out=s, in0=iota_b, scalar1=headsf_t[jt], scalar2=wr_t[jt],
                    op0=Alu.is_equal, op1=Alu.mult,
                )
                soh.append(s)

            # ---- stage 2 ----
            comp_ps = pscomp.tile([128, S], f32)
            mm_stage(
                64, w2_sb, 32, hT,
                lambda m: tokT8_sb[m // 4][:, (m % 4) * 512 : (m % 4 + 1) * 512],
                16, comp_ps, 16, "prod2",
            )

            specT = work.tile([128, S], f16, name="specT", tag="specT")
            nc.scalar.activation(specT, comp_ps, Act.Tanh, bias=bcomp_c)
            deltaT = keep.tile([128, S], f16)
            nc.vector.tensor_scalar(
                out=deltaT, in0=specT, scalar1=base_c, scalar2=None,
                op0=Alu.subtract,
            )

            # transpose deltaT -> delta[j, o] per token tile, build soh, final
            delta_sb = []
            for jt in range(JT):
                dps = psmm.tile([128, 512], f16, name="mm", tag="mm")
                nc.tensor.transpose(
                    dps[:, 0:128], deltaT[:, jt * 128 : (jt + 1) * 128], ident16
                )
                dsb = keep.tile([128, 128], f16, name=f"delta{jt}", tag=f"delta{jt}")
                nc.scalar.copy(dsb, dps[:, 0:128])
                delta_sb.append(dsb)

            fin_ps = psfin.tile([128, S], f32)
            for ic in range(2):
                for jt in range(JT):
                    nc.tensor.matmul(
                        fin_ps[:, ic * 128 : (ic + 1) * 128],
                        soh[jt][:, ic * 128 : (ic + 1) * 128],
                        delta_sb[jt],
                        start=(jt == 0),
                        stop=(jt == JT - 1),
                    )
            for ic in range(2):
                outsb = work.tile([128, T], f32, name="outsb", tag="outsb")
                nc.vector.tensor_add(
                    outsb, fin_ps[:, ic * 128 : (ic + 1) * 128], c0_b
                )
                nc.sync.dma_start(
                    out=out_d[ic * 128 : (ic + 1) * 128, :], in_=outsb
                )

    _NC_CACHE["nc"] = nc
    return nc


def prep_core_inputs(token_embeddings, dep_embeddings, dep_heads,
                     W_dep, b_dep, W_comp, b_comp, W_red, b_red):
    f32 = np.float32
    f16 = np.float16
    tok = np.asarray(token_embeddings, dtype=f32)
    dep = np.asarray(dep_embeddings, dtype=f32)
    heads = np.asarray(dep_heads)
    W_dep = np.asarray(W_dep, dtype=f32)
    b_dep = np.asarray(b_dep, dtype=f32)
    W_comp = np.asarray(W_comp, dtype=f32)
    b_comp = np.asarray(b_comp, dtype=f32)
    wr = np.asarray(W_red, dtype=f32)[0]
    b_red = np.asarray(b_red, dtype=f32)

    # w1t[(a,b), t, (p'*8+d')] = W_dep[16a+p', t, 8b+d']
    X = W_dep.reshape(8, 16, T, 8, 8)            # [a, p', t, b, d']
    w1t = np.ascontiguousarray(
        X.transpose(2, 0, 3, 1, 4).reshape(T, 64 * 128)
    ).astype(f16)                                # [t, ((a,b), (p',d'))]
    # w2t[(a,b), p, (o'*8+t')] = W_comp[16a+o', 8b+t', p]
    Y = W_comp.reshape(8, 16, 16, 8, P)          # [a, o', b, t', p]
    w2t = np.ascontiguousarray(
        Y.transpose(4, 0, 2, 1, 3).reshape(P, 128 * 128)
    ).astype(f16)                                # [p, ((a,b), (o',t'))]
    # red[par][r, m] = (m == 16*par + r//8)
    r = np.arange(128)
    red = np.zeros((128, 2, 32), dtype=f16)
    for par in range(2):
        red[r, par, 16 * par + r // 8] = 1.0
    red = red.reshape(128, 64)

    base = np.tanh(b_comp)
    c0 = (base * wr.sum() + b_red[0]).astype(f32)
    headsf = heads.astype(f32).reshape(B, JT, 128)
    wr_cols = wr.reshape(JT, 128)

    # cf16: [iota broadcast (S) | red (64)]
    cf16 = np.empty((128, S + 64), dtype=f16)
    cf16[:, 0:S] = np.arange(S, dtype=f16)[None, :]
    cf16[:, S:] = red

    shared = {"w1t": w1t, "w2t": w2t, "cf16": cf16}
    in_maps = []
    for c in range(NCORES):
        tokc = tok[c]                             # [S, T]
        depc = dep[c]                             # [S, D]
        tokTc = np.ascontiguousarray(tokc.T)      # [T, S]
        tokT8 = np.empty((16, 128, S), dtype=f32)
        for b in range(16):
            tokT8[b] = np.tile(tokTc[8 * b : 8 * b + 8, :], (16, 1))
        depT = depc.T                             # [D, S]
        depT8 = np.empty((8, 128, S), dtype=f32)
        for b in range(8):
            depT8[b] = np.tile(depT[8 * b : 8 * b + 8, :], (16, 1))
        m = dict(shared)
        m["tokT"] = tokTc.astype(f16)
        m["tokT8"] = np.ascontiguousarray(
            tokT8.reshape(8, 2, 128, S).transpose(0, 2, 1, 3).reshape(8, 128, 2 * S)
            .transpose(1, 0, 2).reshape(128, 8 * 512)
        ).astype(f16)
        m["depT8"] = np.ascontiguousarray(
            depT8.reshape(4, 2, 128, S).transpose(0, 2, 1, 3).reshape(4, 128, 2 * S)
            .transpose(1, 0, 2).reshape(128, 4 * 512)
        ).astype(f16)
        # cf32: [bdep | bcomp | base | headsf0 | headsf1 | wr0 | wr1 | c0(T)]
        cf32 = np.empty((128, 7 + T), dtype=f32)
        cf32[:, 0] = b_dep
        cf32[:, 1] = b_comp
        cf32[:, 2] = base
        cf32[:, 3] = headsf[c, 0]
        cf32[:, 4] = headsf[c, 1]
        cf32[:, 5] = wr_cols[0]
        cf32[:, 6] = wr_cols[1]
        cf32[:, 7:] = c0[None, :]
        m["cf32"] = cf32
        in_maps.append(m)
    return in_maps


def kernel(**inputs) -> np.ndarray:
    _install_compat()
    from concourse.bass_utils import run_bass_kernel_spmd

    nc = build_nc()
    in_maps = prep_core_inputs(**inputs)
    res = run_bass_kernel_spmd(nc, in_maps, core_ids=list(range(NCORES)))
    out = np.stack([res.results[c]["out"] for c in range(NCORES)], axis=0)
    return out.astype(np.float32)


# aliases used by test harness
_build_nc = build_nc
_prep_core_inputs = prep_core_inputs
